# revision 18
# baseline (speedup 1.0000x reference)
"""Trainium2 Bass kernel for nn_Hankel (MPS chain over encoded trajectory).

Math (per sample b):
  h   = relu(x @ W1.T + b1)            [T, HID]
  enc = relu(h @ W2.T + b2)            [T, ENC]
  v0  = enc[0] @ H_first[0]            [R]
  for t in 0..T-3:  M_t = einsum('e,per->pr', enc[t+1], H_mid[t]); v = v @ M_t
  out = v @ (enc[T-1] @ H_last[:,:,0].T)   scalar

Strategy: pure data parallel over 8 cores (1024 samples each).
Per core, per 128-sample tile:
  - encoder as PE matmuls (weights stationary), relu evacuations split
    between the Scalar (ACT) and Vector (DVE) engines
  - M_t formed as matmul: stationary enc_t^T [e=128, b=128],
    moving H_mid[t] host-permuted to [e, (r p)]; out PSUM [b, (r p)] fp32
  - the per-sample p-contraction v'[b,r] = sum_p M[b,(r p)] v[b,p] runs as
    ONE custom DVE instruction: a segmented multiply-accumulate scan
    (registered at import into the concourse custom-DVE table machinery,
    with a hand-authored 2X_1P uop program for fp16 SBUF operands).
    Six of eight b-tiles evacuate PSUM->SBUF fp16 on ACT then scan at 2x;
    the other two scan the fp32 PSUM directly at 1x, freeing ACT.
  - v0 / last-core contraction batched across all 8 tiles per PSUM tile.
"""

import sys

for _p in ("/opt/trn_rl_repo", "/root/.axon_site/_ro/trn_rl_repo"):
    if _p not in sys.path:
        sys.path.append(_p)

import numpy as np
import ml_dtypes

B, T, D, HID, ENC, R = 8192, 12, 64, 512, 128, 64
NCORES = 8
BC = B // NCORES          # samples per core
NTILES = BC // 128        # 8 tiles of 128 samples
BT = BC * T               # 12288 (t-major: col = t*BC + b)
NCHUNK = BT // 512        # 24 encoder n-chunks
NPA = 6                   # b-tiles per t taking the ACT-evac (2x scan) path
F16NP = np.float16
# The MPS chain decays ~80x per step; rescale H tensors by 2^6 (exact in fp)
# so fp16 intermediates stay in range, and unscale the output on host.
SCALE = 64.0
NSCALED = 12              # Hf + 10*Hm + Hl each carry one factor of SCALE

_CACHE = {}


# --------------------------------------------------------------------------
# Custom DVE op: segmented multiply-accumulate scan.
#   out[p, s, k] = sum_{j<=k} in0[p, s, j] * in1[p, s, j]   (reset at each s)
# Lowered via the stock Spec machinery (scan + a patched step-state that
# re-seeds the accumulator at SUB_DIM_DONE), plus a hand-authored 2X_1P
# program (pair-granular; odd positions carry the true inclusive prefix --
# only position N-1 is consumed).
# --------------------------------------------------------------------------

def _register_segscan():
    if "op" in _CACHE:
        return _CACHE["op"]

    from concourse import dve_spec as _ds
    from concourse import dve_ops as _do
    from concourse.dve_spec import (
        AluOp, Scan, Spec, Src0, Src1, Zero, lower, _Stage, _node_as_stage,
    )
    from concourse.dve_uop import (
        DveOpSpec, UopConfig, AluInp, DelayInp, InpSel, OutPath, OutSel,
        Trigger, ENABLE,
    )

    class SegScan(Scan):
        pass

    def _patched_scan_overrides(scans, node_stage):
        seed, step = {}, {}
        for scan in scans:
            d = node_stage[scan]
            init = scan.init if scan.init is not None else _ds._ACCUM_IDENTITY[scan.op]
            seed[d] = _node_as_stage(init)
            if isinstance(scan, SegScan):
                step[d] = _Stage(AluOp.BYPASS, scan.expr)
            elif scan._subdim_step is not None:
                step[d] = _Stage(scan.op, _ds.AluInp.CURR_ALU_OUT, scan._subdim_step)
        return seed, step

    _ds._scan_overrides = _patched_scan_overrides

    def _seg_mac_ref(in0, in1, s0, s1, imm2):
        a = np.asarray(in0, np.float32)
        b = np.asarray(in1, np.float32)
        prod = (a * b).reshape(a.shape[0], -1, a.shape[-1])
        return np.cumsum(prod, axis=-1).reshape(a.shape)

    def _build_2x_uops():
        def mk_body(seg_reset):
            u = UopConfig()
            u.enable_input(InpSel.SRC_0, 0)
            u.enable_input(InpSel.SRC_1, 1)
            u.enable_input(InpSel.SRC_0_HI, 2)
            u.enable_input(InpSel.SRC_1_HI, 3)
            dp = u.datapath_config
            dp[0].enable_alu(AluOp.MULTIPLY, AluInp.PREV_ALU_OUT, AluInp.PREV_DELAY_0)
            dp[0].enable_delay_from_src(DelayInp.PREV_DELAY, 1)
            dp[0].enable_delay_from_src(DelayInp.PREV_DELAY, 2)
            dp[1].enable_alu(AluOp.MULTIPLY, AluInp.PREV_DELAY_1, AluInp.PREV_DELAY_2)
            dp[1].enable_delay_from_src(DelayInp.PREV_ALU_OUT, 0)
            dp[2].enable_alu(AluOp.ADD, AluInp.PREV_ALU_OUT, AluInp.PREV_DELAY_0)
            if seg_reset:
                dp[3].enable_alu(AluOp.BYPASS, AluInp.PREV_ALU_OUT, AluInp.PREV_ALU_OUT)
            else:
                dp[3].enable_alu(AluOp.ADD, AluInp.CURR_ALU_OUT, AluInp.PREV_ALU_OUT)
            for b in range(4, 8):
                dp[b].pass_through_alu()
            u.enable_output(OutSel.ALU_OUT, OutPath.WR0_LO)
            u.enable_output(OutSel.ALU_OUT, OutPath.WR0_HI)
            u.require_inp0 = ENABLE
            u.require_inp1 = ENABLE
            return u

        seed = UopConfig()
        seed.enable_input(InpSel.ZERO, 0)
        for b in range(0, 8):
            seed.datapath_config[b].pass_through_alu()
        seed.datapath_config[3].enable_alu(
            AluOp.BYPASS, AluInp.PREV_ALU_OUT, AluInp.PREV_ALU_OUT
        )
        seed.trigger = (Trigger.COUNT, Trigger.NONE, Trigger.NONE)
        seed.repeat_count = 1
        seed.next_uop = (1, 0, 0)

        steady = mk_body(False)
        steady.trigger = (Trigger.SRC_TENSOR_DONE, Trigger.SUB_DIM_DONE, Trigger.NONE)
        steady.next_uop = (0, 2, 0)

        step = mk_body(True)
        step.trigger = (Trigger.SRC_TENSOR_DONE, Trigger.SUB_DIM_DONE, Trigger.COUNT)
        step.repeat_count = 1
        step.next_uop = (0, 2, 1)
        return [seed, steady, step]

    body = SegScan(AluOp.ADD, Src0 * Src1, init=Zero)
    spec = Spec(body=body, reference=_seg_mac_ref)

    name = "SEG_MAC_SCAN_ANT"
    if name not in _do._SUB_OPCODE_FOR_NAME:
        _do._SUB_OPCODE_FOR_NAME[name] = max(_do._SUB_OPCODE_FOR_NAME.values()) + 1
    row = _do._SUB_OPCODE_FOR_NAME[name]
    assert row < 0x20

    shas = {}
    for ver in ("v3", "v4"):
        ds = DveOpSpec(name=name, opcode=row, uops=lower(spec, ver=ver),
                       uops_2x=_build_2x_uops(), rd1_en=True)
        shas[ver] = ds.sha(ver)

    class DveOp2x(_do.DveOp):
        def compile(self, ver):
            key = (self.name, ver)
            cached = _do._COMPILE_CACHE.get(key)
            if cached is not None:
                return cached
            result = DveOpSpec(
                name=self.name, opcode=_do.get_dve_sub_opcode(self.name),
                uops=lower(self.spec, ver=ver), uops_2x=_build_2x_uops(),
                rd1_en=True,
            )
            got = result.sha(ver)
            if self.uops_sha.get(ver) != got:
                raise ValueError(f"{self.name}: sha drift {ver}: {got}")
            _do._COMPILE_CACHE[key] = result
            return result

    # inject perf_max=1 (byte-36[7:6]) at construction so the engine may
    # reach the 2X_1P slot when operands qualify (fp16, step 1, aligned).
    from concourse import bass_isa as _bisa

    _real_inst = _bisa.InstCustomDveAnt

    def _patched_inst(*a, **kw):
        if kw.get("op_name") == name:
            kw.setdefault("perf_max", 1)
        return _real_inst(*a, **kw)

    _bisa.InstCustomDveAnt = _patched_inst

    op = DveOp2x(name, spec, subdim=True, uops_sha=shas)
    if all(o.name != name for o in _do.OPS):
        _do.OPS.append(op)
    _do.CUSTOM_DVE_SPECS[name] = spec
    _CACHE["op"] = op
    return op


def _build():
    import concourse.bass as bass
    import concourse.tile as tile
    from concourse import bacc, mybir
    from contextlib import ExitStack

    F16 = mybir.dt.float16
    F32 = mybir.dt.float32
    AX = mybir.AxisListType
    OP = mybir.AluOpType
    AF = mybir.ActivationFunctionType

    segop = _register_segscan()

    nc = bacc.Bacc(None, target_bir_lowering=False, debug=False)

    # xT rows 0-63 = x^T, rows 64-127 duplicate, so two K=64 row-tiles of
    # the w1 matmul run concurrently on the PE array
    xT = nc.declare_dram_parameter("xT", [128, BT], F16, isOutput=False)
    w1 = nc.declare_dram_parameter("w1", [128, HID // 256, 128], F16, isOutput=False)
    w2 = nc.declare_dram_parameter("w2", [128, HID // 128, ENC], F16, isOutput=False)
    b1c = nc.declare_dram_parameter("b1c", [128, HID // 128], F32, isOutput=False)
    b2c = nc.declare_dram_parameter("b2c", [128, 1], F32, isOutput=False)
    hm = nc.declare_dram_parameter("hm", [T - 2, ENC, R * R], F16, isOutput=False)
    hf = nc.declare_dram_parameter("hf", [ENC, R], F16, isOutput=False)
    hl = nc.declare_dram_parameter("hl", [ENC, R], F16, isOutput=False)
    out = nc.declare_dram_parameter("out", [128, NTILES], F32, isOutput=True)

    with tile.TileContext(nc) as tc, ExitStack() as ctx:
        const = ctx.enter_context(tc.tile_pool(name="const", bufs=1))
        hbuf = ctx.enter_context(tc.tile_pool(name="hbuf", bufs=2))
        hwork = ctx.enter_context(tc.tile_pool(name="hwork", bufs=2))
        mwork = ctx.enter_context(tc.tile_pool(name="mwork", bufs=2))
        owork = ctx.enter_context(tc.tile_pool(name="owork", bufs=2))

        # ---- load constants / inputs to SBUF ----
        xT_sb = const.tile([128, BT], F16)
        nc.sync.dma_start(out=xT_sb[:], in_=xT[:])
        w1_sb = const.tile([128, HID // 256, 128], F16)
        nc.sync.dma_start(out=w1_sb[:], in_=w1[:])
        w2_sb = const.tile([128, HID // 128, ENC], F16)
        nc.sync.dma_start(out=w2_sb[:], in_=w2[:])
        b1c_sb = const.tile([128, HID // 128], F32)
        nc.sync.dma_start(out=b1c_sb[:], in_=b1c[:])
        b2c_sb = const.tile([128, 1], F32)
        nc.sync.dma_start(out=b2c_sb[:], in_=b2c[:])
        hf_sb = const.tile([ENC, R], F16)
        nc.sync.dma_start(out=hf_sb[:], in_=hf[:])
        hl_sb = const.tile([ENC, R], F16)
        nc.sync.dma_start(out=hl_sb[:], in_=hl[:])

        encT_sb = const.tile([ENC, BT], F16)     # [e, t*BC + b]
        v_sb = const.tile([128, NTILES, R], F16)
        last_sb = const.tile([128, NTILES, R], F16)
        out_sb = const.tile([128, NTILES], F32)

        # ================= phase 1: encoder + v0 + last =================
        with tc.tile_pool(name="ps_a", bufs=2, space="PSUM") as ps_a, \
             tc.tile_pool(name="ps_b", bufs=2, space="PSUM") as ps_b, \
             tc.tile_pool(name="ps_s", bufs=1, space="PSUM") as ps_s:
            NH = HID // 128  # 4 hid chunks
            for n in range(NCHUNK):
                # alternate the whole n-chunk's bias+relu evacuations between
                # the Scalar and Vector engines (chunk-level, to avoid per-op
                # cross-engine sync on the shared psum tiles)
                on_act = (n % 2 == 0)
                ncol = slice(n * 512, (n + 1) * 512)
                h_sb = hwork.tile([128, NH, 512], F16, tag="h_sb")
                for pair in range(NH // 2):
                    ps1 = ps_a.tile([128, 2, 512], F32, tag="ps1")
                    # two K=64 row-tiles run concurrently on the PE array
                    nc.tensor.matmul(
                        ps1[:, 0, :], w1_sb[0:64, pair, :], xT_sb[0:64, ncol],
                        tile_position=(0, 0),
                    )
                    nc.tensor.matmul(
                        ps1[:, 1, :], w1_sb[64:128, pair, :], xT_sb[64:128, ncol],
                        tile_position=(64, 0),
                    )
                    for ci in range(2):
                        c = pair * 2 + ci
                        # split the two evacuations of each psum pair across
                        # both engines so neither serializes the chunk
                        if ci == (0 if on_act else 1):
                            nc.scalar.activation(
                                h_sb[:, c, :], ps1[:, ci, :], AF.Relu,
                                bias=b1c_sb[:, c:c + 1],
                            )
                        else:
                            nc.vector.tensor_scalar(
                                h_sb[:, c, :], ps1[:, ci, :],
                                b1c_sb[:, c:c + 1], 0.0,
                                op0=OP.add, op1=OP.max,
                            )
                ps2 = ps_b.tile([128, 512], F32, tag="ps2")
                for c in range(NH):
                    nc.tensor.matmul(
                        ps2[:], w2_sb[:, c, :], h_sb[:, c, :],
                        start=(c == 0), stop=(c == NH - 1),
                    )
                if on_act:
                    nc.vector.tensor_scalar(
                        encT_sb[:, ncol], ps2[:], b2c_sb[:, 0:1], 0.0,
                        op0=OP.add, op1=OP.max,
                    )
                else:
                    nc.scalar.activation(
                        encT_sb[:, ncol], ps2[:], AF.Relu, bias=b2c_sb[:, 0:1],
                    )

            # ---- v0 = enc_0 @ H_first, all 8 tiles into one PSUM tile ----
            psv = ps_s.tile([128, NTILES * R], F32, tag="psv")
            for it in range(NTILES):
                bcol = slice(it * 128, (it + 1) * 128)  # t=0 block
                nc.tensor.matmul(psv[:, it * R:(it + 1) * R],
                                 encT_sb[:, bcol], hf_sb[:])
            nc.scalar.activation(
                v_sb[:].rearrange("b i r -> b (i r)"), psv[:], AF.Copy)

            # ---- last = enc_{T-1} @ H_last, batched likewise ----
            psl = ps_s.tile([128, NTILES * R], F32, tag="psl")
            for it in range(NTILES):
                bcol = slice((T - 1) * BC + it * 128, (T - 1) * BC + (it + 1) * 128)
                nc.tensor.matmul(psl[:, it * R:(it + 1) * R],
                                 encT_sb[:, bcol], hl_sb[:])
            nc.scalar.activation(
                last_sb[:].rearrange("b i r -> b (i r)"), psl[:], AF.Copy)

        # ================= phase 2: the MPS chain =================
        with tc.tile_pool(name="ps_mm", bufs=1, space="PSUM") as ps_mm:
            for t in range(T - 2):
                h_t = hbuf.tile([ENC, R * R], F16, tag="h_t")
                nc.sync.dma_start(out=h_t[:], in_=hm[t])
                # interleave PSUM-path tiles among ACT-path tiles so the DVE
                # PSUM scans overlap ACT's evacuations instead of queuing
                # after all path-A work
                tile_order = [0, 1, 2, 6, 3, 4, 5, 7]
                for it in tile_order:
                    path_a = it < NPA
                    bcol = slice((t + 1) * BC + it * 128,
                                 (t + 1) * BC + (it + 1) * 128)
                    vbc = v_sb[:, it, :]
                    # chunk layout {2048, 1024, 1024}: big chunk amortizes the
                    # ACT per-op overhead, small ones keep PSUM elastic
                    CH = (2048, 1024, 1024)
                    psm = [
                        ps_mm.tile([128, CH[c]], F32, tag=f"psm{c}",
                                   name=f"psm{c}")
                        for c in range(3)
                    ]
                    off = 0
                    for c in range(3):
                        for jj in range(CH[c] // 512):
                            nc.tensor.matmul(
                                psm[c][:, jj * 512:(jj + 1) * 512],
                                encT_sb[:, bcol],
                                h_t[:, off:off + 512],
                            )
                            off += 512
                    if path_a:
                        # ACT evacuates fp32->fp16; DVE scans at 2x
                        m_sb = mwork.tile([128, R * R], F16, tag="m_sb")
                        off = 0
                        for c in range(3):
                            nc.scalar.activation(
                                m_sb[:, off:off + CH[c]], psm[c][:], AF.Copy,
                            )
                            off += CH[c]
                        o3 = owork.tile([128, R, R], F16, tag="o3")
                        nc.vector._custom_dve(
                            segop,
                            out=o3[:],
                            in0=m_sb[:].rearrange("b (r p) -> b r p", p=R),
                            in1=vbc.unsqueeze(1).broadcast_to([128, R, R]),
                        )
                        nc.vector.tensor_copy(
                            v_sb[:, it, :].unsqueeze(2), o3[:, :, R - 1:R])
                    else:
                        # DVE scans the fp32 PSUM directly (1x), ACT idle.
                        # The scans write disjoint r-slices of one output
                        # tile; the single extract runs after all of them
                        # (it also carries the WAR edge protecting v_sb).
                        o3 = owork.tile([128, R, R], F16, tag="o3p")
                        roff = 0
                        for c in range(3):
                            nr = CH[c] // R
                            nc.vector._custom_dve(
                                segop,
                                out=o3[:, roff:roff + nr, :],
                                in0=psm[c][:].rearrange("b (r p) -> b r p", p=R),
                                in1=vbc.unsqueeze(1).broadcast_to([128, nr, R]),
                            )
                            roff += nr
                        nc.vector.tensor_copy(
                            v_sb[:, it, :].unsqueeze(2), o3[:, :, R - 1:R])

            # ---- final: dot(v, last) ----
            for it in range(NTILES):
                prod = hwork.tile([128, R], F32, tag="prod")
                nc.vector.tensor_tensor(
                    out=prod[:], in0=last_sb[:, it, :], in1=v_sb[:, it, :],
                    op=OP.mult,
                )
                nc.vector.tensor_reduce(
                    out_sb[:, it:it + 1], prod[:], axis=AX.X, op=OP.add
                )

            nc.sync.dma_start(out=out[:], in_=out_sb[:])

    nc.compile()
    return nc


def _prep_inputs(x, W1, b1, W2, b2, H_first, H_mid, H_last):
    """Host-side prep: shard x, transpose/permute/cast weights."""
    ins = []
    # w1 packed for two concurrent K=64 row-tiles:
    #   w1h[0:64,  pair, j] = W1.T[:, (2*pair)*128 + j]
    #   w1h[64:128, pair, j] = W1.T[:, (2*pair+1)*128 + j]
    w1t = W1.T.reshape(D, HID // 256, 2, 128)      # [64, pair, ci, 128]
    w1h = np.concatenate([w1t[:, :, 0, :], w1t[:, :, 1, :]], axis=0).astype(F16NP)
    # w2 pre-chunked: w2h[p, c, e] = W2[e, c*128 + p]
    w2h = np.ascontiguousarray(
        W2.T.reshape(HID // 128, 128, ENC).transpose(1, 0, 2)
    ).astype(F16NP)
    b1ch = np.ascontiguousarray(
        b1.reshape(HID // 128, 128).T).astype(np.float32)   # [128, 4]
    b2ch = b2[:, None].astype(np.float32)          # [128, 1]
    # H_mid[t, p, e, r] -> hm[t, e, (r p)] : hm[t,e,r,p] = H_mid[t,p,e,r]
    hmh = (np.ascontiguousarray(np.transpose(H_mid, (0, 2, 3, 1))).reshape(
        T - 2, ENC, R * R
    ) * SCALE).astype(F16NP)
    hfh = (H_first[0] * SCALE).astype(F16NP)       # [ENC, R]
    hlh = (np.ascontiguousarray(H_last[:, :, 0].T) * SCALE).astype(F16NP)
    for c in range(NCORES):
        xs = x[c * BC:(c + 1) * BC]                # [BC, T, D]
        # xT[d, t*BC + b] = x[b, t, d]; rows 64-127 duplicate rows 0-63
        xTh = np.empty((128, BT), dtype=F16NP)
        xTh[:D] = np.transpose(xs, (2, 1, 0)).reshape(D, BT)
        xTh[D:] = xTh[:D]
        ins.append({
            "xT": xTh, "w1": w1h, "w2": w2h, "b1c": b1ch, "b2c": b2ch,
            "hm": hmh, "hf": hfh, "hl": hlh,
        })
    return ins


def kernel(x, W1, b1, W2, b2, H_first, H_mid, H_last):
    from concourse.bass_utils import run_bass_kernel_spmd

    if "nc" not in _CACHE:
        _CACHE["nc"] = _build()
    nc = _CACHE["nc"]

    in_maps = _prep_inputs(x, W1, b1, W2, b2, H_first, H_mid, H_last)
    res = run_bass_kernel_spmd(nc, in_maps, core_ids=list(range(NCORES)))
    # out[b_in_tile, tile] per core -> flat [BC] with index tile*128 + b
    outs = [
        np.asarray(res.results[c]["out"]).T.reshape(BC) for c in range(NCORES)
    ]
    full = np.concatenate(outs, axis=0).astype(np.float64)
    return (full / SCALE**NSCALED).astype(np.float32)


# revision 20
# speedup vs baseline: 1.1740x; 1.1740x over previous
"""Trainium2 Bass kernel for nn_Hankel (MPS chain over encoded trajectory).

Math (per sample b):
  h   = relu(x @ W1.T + b1)            [T, HID]
  enc = relu(h @ W2.T + b2)            [T, ENC]
  v0  = enc[0] @ H_first[0]            [R]
  for t in 0..T-3:  M_t = einsum('e,per->pr', enc[t+1], H_mid[t]); v = v @ M_t
  out = v @ (enc[T-1] @ H_last[:,:,0].T)   scalar

Strategy: pure data parallel over 8 cores (1024 samples each).
Per core, per 128-sample tile:
  - encoder as PE matmuls (weights stationary), relu evacuations split
    between the Scalar (ACT) and Vector (DVE) engines
  - M_t formed as matmul: stationary enc_t^T [e=128, b=128],
    moving H_mid[t] host-permuted to [e, (r p)]; out PSUM [b, (r p)] fp32
  - the per-sample p-contraction v'[b,r] = sum_p M[b,(r p)] v[b,p] runs as
    ONE custom DVE instruction: a segmented multiply-accumulate scan
    (registered at import into the concourse custom-DVE table machinery,
    with a hand-authored 2X_1P uop program for fp16 SBUF operands).
    Six of eight b-tiles evacuate PSUM->SBUF fp16 on ACT then scan at 2x;
    the other two scan the fp32 PSUM directly at 1x, freeing ACT.
  - v0 / last-core contraction batched across all 8 tiles per PSUM tile.
"""

import sys

for _p in ("/opt/trn_rl_repo", "/root/.axon_site/_ro/trn_rl_repo"):
    if _p not in sys.path:
        sys.path.append(_p)

import numpy as np
import ml_dtypes

B, T, D, HID, ENC, R = 8192, 12, 64, 512, 128, 64
NCORES = 8
BC = B // NCORES          # samples per core
NTILES = BC // 128        # 8 tiles of 128 samples
BT = BC * T               # 12288 (t-major: col = t*BC + b)
NCHUNK = BT // 512        # 24 encoder n-chunks
NPA = 6                   # b-tiles per t taking the ACT-evac (2x scan) path
F16NP = np.float16
# The MPS chain decays ~80x per step; rescale H tensors by 2^6 (exact in fp)
# so fp16 intermediates stay in range, and unscale the output on host.
SCALE = 64.0
NSCALED = 12              # Hf + 10*Hm + Hl each carry one factor of SCALE

_CACHE = {}


# --------------------------------------------------------------------------
# Custom DVE op: segmented multiply-accumulate scan.
#   out[p, s, k] = sum_{j<=k} in0[p, s, j] * in1[p, s, j]   (reset at each s)
# Lowered via the stock Spec machinery (scan + a patched step-state that
# re-seeds the accumulator at SUB_DIM_DONE), plus a hand-authored 2X_1P
# program (pair-granular; odd positions carry the true inclusive prefix --
# only position N-1 is consumed).
# --------------------------------------------------------------------------

def _register_segscan():
    if "op" in _CACHE:
        return _CACHE["op"]

    from concourse import dve_spec as _ds
    from concourse import dve_ops as _do
    from concourse.dve_spec import (
        AluOp, Scan, Spec, Src0, Src1, Zero, lower, _Stage, _node_as_stage,
    )
    from concourse.dve_uop import (
        DveOpSpec, UopConfig, AluInp, DelayInp, InpSel, OutPath, OutSel,
        Trigger, ENABLE,
    )

    class SegScan(Scan):
        pass

    def _patched_scan_overrides(scans, node_stage):
        seed, step = {}, {}
        for scan in scans:
            d = node_stage[scan]
            init = scan.init if scan.init is not None else _ds._ACCUM_IDENTITY[scan.op]
            seed[d] = _node_as_stage(init)
            if isinstance(scan, SegScan):
                step[d] = _Stage(AluOp.BYPASS, scan.expr)
            elif scan._subdim_step is not None:
                step[d] = _Stage(scan.op, _ds.AluInp.CURR_ALU_OUT, scan._subdim_step)
        return seed, step

    _ds._scan_overrides = _patched_scan_overrides

    def _seg_mac_ref(in0, in1, s0, s1, imm2):
        a = np.asarray(in0, np.float32)
        b = np.asarray(in1, np.float32)
        prod = (a * b).reshape(a.shape[0], -1, a.shape[-1])
        return np.cumsum(prod, axis=-1).reshape(a.shape)

    def _build_2x_uops():
        def mk_body(seg_reset):
            u = UopConfig()
            u.enable_input(InpSel.SRC_0, 0)
            u.enable_input(InpSel.SRC_1, 1)
            u.enable_input(InpSel.SRC_0_HI, 2)
            u.enable_input(InpSel.SRC_1_HI, 3)
            dp = u.datapath_config
            dp[0].enable_alu(AluOp.MULTIPLY, AluInp.PREV_ALU_OUT, AluInp.PREV_DELAY_0)
            dp[0].enable_delay_from_src(DelayInp.PREV_DELAY, 1)
            dp[0].enable_delay_from_src(DelayInp.PREV_DELAY, 2)
            dp[1].enable_alu(AluOp.MULTIPLY, AluInp.PREV_DELAY_1, AluInp.PREV_DELAY_2)
            dp[1].enable_delay_from_src(DelayInp.PREV_ALU_OUT, 0)
            dp[2].enable_alu(AluOp.ADD, AluInp.PREV_ALU_OUT, AluInp.PREV_DELAY_0)
            if seg_reset:
                dp[3].enable_alu(AluOp.BYPASS, AluInp.PREV_ALU_OUT, AluInp.PREV_ALU_OUT)
            else:
                dp[3].enable_alu(AluOp.ADD, AluInp.CURR_ALU_OUT, AluInp.PREV_ALU_OUT)
            for b in range(4, 8):
                dp[b].pass_through_alu()
            u.enable_output(OutSel.ALU_OUT, OutPath.WR0_LO)
            u.enable_output(OutSel.ALU_OUT, OutPath.WR0_HI)
            u.require_inp0 = ENABLE
            u.require_inp1 = ENABLE
            return u

        seed = UopConfig()
        seed.enable_input(InpSel.ZERO, 0)
        for b in range(0, 8):
            seed.datapath_config[b].pass_through_alu()
        seed.datapath_config[3].enable_alu(
            AluOp.BYPASS, AluInp.PREV_ALU_OUT, AluInp.PREV_ALU_OUT
        )
        seed.trigger = (Trigger.COUNT, Trigger.NONE, Trigger.NONE)
        seed.repeat_count = 1
        seed.next_uop = (1, 0, 0)

        steady = mk_body(False)
        steady.trigger = (Trigger.SRC_TENSOR_DONE, Trigger.SUB_DIM_DONE, Trigger.NONE)
        steady.next_uop = (0, 2, 0)

        step = mk_body(True)
        step.trigger = (Trigger.SRC_TENSOR_DONE, Trigger.SUB_DIM_DONE, Trigger.COUNT)
        step.repeat_count = 1
        step.next_uop = (0, 2, 1)
        return [seed, steady, step]

    body = SegScan(AluOp.ADD, Src0 * Src1, init=Zero)
    spec = Spec(body=body, reference=_seg_mac_ref)

    name = "SEG_MAC_SCAN_ANT"
    if name not in _do._SUB_OPCODE_FOR_NAME:
        _do._SUB_OPCODE_FOR_NAME[name] = max(_do._SUB_OPCODE_FOR_NAME.values()) + 1
    row = _do._SUB_OPCODE_FOR_NAME[name]
    assert row < 0x20

    shas = {}
    for ver in ("v3", "v4"):
        ds = DveOpSpec(name=name, opcode=row, uops=lower(spec, ver=ver),
                       uops_2x=_build_2x_uops(), rd1_en=True)
        shas[ver] = ds.sha(ver)

    class DveOp2x(_do.DveOp):
        def compile(self, ver):
            key = (self.name, ver)
            cached = _do._COMPILE_CACHE.get(key)
            if cached is not None:
                return cached
            result = DveOpSpec(
                name=self.name, opcode=_do.get_dve_sub_opcode(self.name),
                uops=lower(self.spec, ver=ver), uops_2x=_build_2x_uops(),
                rd1_en=True,
            )
            got = result.sha(ver)
            if self.uops_sha.get(ver) != got:
                raise ValueError(f"{self.name}: sha drift {ver}: {got}")
            _do._COMPILE_CACHE[key] = result
            return result

    # inject perf_max=1 (byte-36[7:6]) at construction so the engine may
    # reach the 2X_1P slot when operands qualify (fp16, step 1, aligned).
    from concourse import bass_isa as _bisa

    _real_inst = _bisa.InstCustomDveAnt

    def _patched_inst(*a, **kw):
        if kw.get("op_name") == name:
            kw.setdefault("perf_max", 1)
        return _real_inst(*a, **kw)

    _bisa.InstCustomDveAnt = _patched_inst

    op = DveOp2x(name, spec, subdim=True, uops_sha=shas)
    if all(o.name != name for o in _do.OPS):
        _do.OPS.append(op)
    _do.CUSTOM_DVE_SPECS[name] = spec
    _CACHE["op"] = op
    return op


def _build():
    import concourse.bass as bass
    import concourse.tile as tile
    from concourse import bacc, mybir
    from contextlib import ExitStack

    F16 = mybir.dt.float16
    F32 = mybir.dt.float32
    AX = mybir.AxisListType
    OP = mybir.AluOpType
    AF = mybir.ActivationFunctionType

    segop = _register_segscan()

    nc = bacc.Bacc(None, target_bir_lowering=False, debug=False)

    # xT rows 0-63 = x^T, rows 64-127 duplicate, so two K=64 row-tiles of
    # the w1 matmul run concurrently on the PE array
    xT = nc.declare_dram_parameter("xT", [128, BT], F16, isOutput=False)
    w1 = nc.declare_dram_parameter("w1", [128, HID // 256, 128], F16, isOutput=False)
    w2 = nc.declare_dram_parameter("w2", [128, HID // 128, ENC], F16, isOutput=False)
    b1c = nc.declare_dram_parameter("b1c", [128, HID // 128], F32, isOutput=False)
    b2c = nc.declare_dram_parameter("b2c", [128, 1], F32, isOutput=False)
    hm = nc.declare_dram_parameter("hm", [T - 2, ENC, R * R], F16, isOutput=False)
    hf = nc.declare_dram_parameter("hf", [ENC, R], F16, isOutput=False)
    hl = nc.declare_dram_parameter("hl", [ENC, R], F16, isOutput=False)
    out = nc.declare_dram_parameter("out", [128, NTILES], F32, isOutput=True)

    with tile.TileContext(nc) as tc, ExitStack() as ctx:
        const = ctx.enter_context(tc.tile_pool(name="const", bufs=1))
        hbuf = ctx.enter_context(tc.tile_pool(name="hbuf", bufs=2))
        hwork = ctx.enter_context(tc.tile_pool(name="hwork", bufs=2))
        mwork = ctx.enter_context(tc.tile_pool(name="mwork", bufs=2))
        owork = ctx.enter_context(tc.tile_pool(name="owork", bufs=2))

        # ---- load constants / inputs to SBUF ----
        xT_sb = const.tile([128, BT], F16)
        nc.sync.dma_start(out=xT_sb[:], in_=xT[:])
        w1_sb = const.tile([128, HID // 256, 128], F16)
        nc.sync.dma_start(out=w1_sb[:], in_=w1[:])
        w2_sb = const.tile([128, HID // 128, ENC], F16)
        nc.sync.dma_start(out=w2_sb[:], in_=w2[:])
        b1c_sb = const.tile([128, HID // 128], F32)
        nc.sync.dma_start(out=b1c_sb[:], in_=b1c[:])
        b2c_sb = const.tile([128, 1], F32)
        nc.sync.dma_start(out=b2c_sb[:], in_=b2c[:])
        hf_sb = const.tile([ENC, R], F16)
        nc.sync.dma_start(out=hf_sb[:], in_=hf[:])
        hl_sb = const.tile([ENC, R], F16)
        nc.sync.dma_start(out=hl_sb[:], in_=hl[:])

        encT_sb = const.tile([ENC, BT], F16)     # [e, t*BC + b]
        v_sb = const.tile([128, NTILES, R], F16)
        last_sb = const.tile([128, NTILES, R], F16)
        out_sb = const.tile([128, NTILES], F32)

        # ================= phase 1: encoder + v0 + last =================
        with tc.tile_pool(name="ps_a", bufs=2, space="PSUM") as ps_a, \
             tc.tile_pool(name="ps_b", bufs=2, space="PSUM") as ps_b, \
             tc.tile_pool(name="ps_s", bufs=1, space="PSUM") as ps_s:
            NH = HID // 128  # 4 hid chunks
            for n in range(NCHUNK):
                # alternate the whole n-chunk's bias+relu evacuations between
                # the Scalar and Vector engines (chunk-level, to avoid per-op
                # cross-engine sync on the shared psum tiles)
                on_act = (n % 2 == 0)
                ncol = slice(n * 512, (n + 1) * 512)
                h_sb = hwork.tile([128, NH, 512], F16, tag="h_sb")
                for pair in range(NH // 2):
                    ps1 = ps_a.tile([128, 2, 512], F32, tag="ps1")
                    # two K=64 row-tiles run concurrently on the PE array
                    nc.tensor.matmul(
                        ps1[:, 0, :], w1_sb[0:64, pair, :], xT_sb[0:64, ncol],
                        tile_position=(0, 0),
                    )
                    nc.tensor.matmul(
                        ps1[:, 1, :], w1_sb[64:128, pair, :], xT_sb[64:128, ncol],
                        tile_position=(64, 0),
                    )
                    for ci in range(2):
                        c = pair * 2 + ci
                        # split the two evacuations of each psum pair across
                        # both engines so neither serializes the chunk
                        if ci == (0 if on_act else 1):
                            nc.scalar.activation(
                                h_sb[:, c, :], ps1[:, ci, :], AF.Relu,
                                bias=b1c_sb[:, c:c + 1],
                            )
                        else:
                            nc.vector.tensor_scalar(
                                h_sb[:, c, :], ps1[:, ci, :],
                                b1c_sb[:, c:c + 1], 0.0,
                                op0=OP.add, op1=OP.max,
                            )
                ps2 = ps_b.tile([128, 512], F32, tag="ps2")
                for c in range(NH):
                    nc.tensor.matmul(
                        ps2[:], w2_sb[:, c, :], h_sb[:, c, :],
                        start=(c == 0), stop=(c == NH - 1),
                    )
                if on_act:
                    nc.vector.tensor_scalar(
                        encT_sb[:, ncol], ps2[:], b2c_sb[:, 0:1], 0.0,
                        op0=OP.add, op1=OP.max,
                    )
                else:
                    nc.scalar.activation(
                        encT_sb[:, ncol], ps2[:], AF.Relu, bias=b2c_sb[:, 0:1],
                    )

            # ---- v0 = enc_0 @ H_first, all 8 tiles into one PSUM tile ----
            psv = ps_s.tile([128, NTILES * R], F32, tag="psv")
            for it in range(NTILES):
                bcol = slice(it * 128, (it + 1) * 128)  # t=0 block
                nc.tensor.matmul(psv[:, it * R:(it + 1) * R],
                                 encT_sb[:, bcol], hf_sb[:])
            nc.scalar.activation(
                v_sb[:].rearrange("b i r -> b (i r)"), psv[:], AF.Copy)

            # ---- last = enc_{T-1} @ H_last, batched likewise ----
            psl = ps_s.tile([128, NTILES * R], F32, tag="psl")
            for it in range(NTILES):
                bcol = slice((T - 1) * BC + it * 128, (T - 1) * BC + (it + 1) * 128)
                nc.tensor.matmul(psl[:, it * R:(it + 1) * R],
                                 encT_sb[:, bcol], hl_sb[:])
            nc.scalar.activation(
                last_sb[:].rearrange("b i r -> b (i r)"), psl[:], AF.Copy)

        # ================= phase 2: the MPS chain =================
        with tc.tile_pool(name="ps_mm", bufs=1, space="PSUM") as ps_mm:
            for t in range(T - 2):
                h_t = hbuf.tile([ENC, R * R], F16, tag="h_t")
                nc.sync.dma_start(out=h_t[:], in_=hm[t])
                # interleave PSUM-path tiles among ACT-path tiles so the DVE
                # PSUM scans overlap ACT's evacuations instead of queuing
                # after all path-A work
                tile_order = [0, 1, 2, 6, 3, 4, 5, 7]
                for it in tile_order:
                    path_a = it < NPA
                    bcol = slice((t + 1) * BC + it * 128,
                                 (t + 1) * BC + (it + 1) * 128)
                    vbc = v_sb[:, it, :]
                    # four equal chunks keep PSUM maximally elastic
                    CH = (1024, 1024, 1024, 1024)
                    psm = [
                        ps_mm.tile([128, CH[c]], F32, tag=f"psm{c}",
                                   name=f"psm{c}")
                        for c in range(len(CH))
                    ]
                    off = 0
                    for c in range(len(CH)):
                        for jj in range(CH[c] // 512):
                            nc.tensor.matmul(
                                psm[c][:, jj * 512:(jj + 1) * 512],
                                encT_sb[:, bcol],
                                h_t[:, off:off + 512],
                            )
                            off += 512
                    if path_a:
                        # ACT evacuates fp32->fp16; DVE scans at 2x
                        m_sb = mwork.tile([128, R * R], F16, tag="m_sb")
                        off = 0
                        for c in range(len(CH)):
                            nc.scalar.activation(
                                m_sb[:, off:off + CH[c]], psm[c][:], AF.Copy,
                            )
                            off += CH[c]
                        o3 = owork.tile([128, R, R], F16, tag="o3")
                        nc.vector._custom_dve(
                            segop,
                            out=o3[:],
                            in0=m_sb[:].rearrange("b (r p) -> b r p", p=R),
                            in1=vbc.unsqueeze(1).broadcast_to([128, R, R]),
                        )
                        nc.vector.tensor_copy(
                            v_sb[:, it, :].unsqueeze(2), o3[:, :, R - 1:R])
                    else:
                        # DVE scans the fp32 PSUM directly (1x), ACT idle.
                        # The scans write disjoint r-slices of one output
                        # tile; the single extract runs after all of them
                        # (it also carries the WAR edge protecting v_sb).
                        o3 = owork.tile([128, R, R], F16, tag="o3p")
                        roff = 0
                        for c in range(len(CH)):
                            nr = CH[c] // R
                            nc.vector._custom_dve(
                                segop,
                                out=o3[:, roff:roff + nr, :],
                                in0=psm[c][:].rearrange("b (r p) -> b r p", p=R),
                                in1=vbc.unsqueeze(1).broadcast_to([128, nr, R]),
                            )
                            roff += nr
                        nc.vector.tensor_copy(
                            v_sb[:, it, :].unsqueeze(2), o3[:, :, R - 1:R])

            # ---- final: dot(v, last) ----
            for it in range(NTILES):
                prod = hwork.tile([128, R], F32, tag="prod")
                nc.vector.tensor_tensor(
                    out=prod[:], in0=last_sb[:, it, :], in1=v_sb[:, it, :],
                    op=OP.mult,
                )
                nc.vector.tensor_reduce(
                    out_sb[:, it:it + 1], prod[:], axis=AX.X, op=OP.add
                )

            nc.sync.dma_start(out=out[:], in_=out_sb[:])

    nc.compile()
    return nc


def _prep_inputs(x, W1, b1, W2, b2, H_first, H_mid, H_last):
    """Host-side prep: shard x, transpose/permute/cast weights."""
    ins = []
    # w1 packed for two concurrent K=64 row-tiles:
    #   w1h[0:64,  pair, j] = W1.T[:, (2*pair)*128 + j]
    #   w1h[64:128, pair, j] = W1.T[:, (2*pair+1)*128 + j]
    w1t = W1.T.reshape(D, HID // 256, 2, 128)      # [64, pair, ci, 128]
    w1h = np.concatenate([w1t[:, :, 0, :], w1t[:, :, 1, :]], axis=0).astype(F16NP)
    # w2 pre-chunked: w2h[p, c, e] = W2[e, c*128 + p]
    w2h = np.ascontiguousarray(
        W2.T.reshape(HID // 128, 128, ENC).transpose(1, 0, 2)
    ).astype(F16NP)
    b1ch = np.ascontiguousarray(
        b1.reshape(HID // 128, 128).T).astype(np.float32)   # [128, 4]
    b2ch = b2[:, None].astype(np.float32)          # [128, 1]
    # H_mid[t, p, e, r] -> hm[t, e, (r p)] : hm[t,e,r,p] = H_mid[t,p,e,r]
    hmh = (np.ascontiguousarray(np.transpose(H_mid, (0, 2, 3, 1))).reshape(
        T - 2, ENC, R * R
    ) * SCALE).astype(F16NP)
    hfh = (H_first[0] * SCALE).astype(F16NP)       # [ENC, R]
    hlh = (np.ascontiguousarray(H_last[:, :, 0].T) * SCALE).astype(F16NP)
    for c in range(NCORES):
        xs = x[c * BC:(c + 1) * BC]                # [BC, T, D]
        # xT[d, t*BC + b] = x[b, t, d]; rows 64-127 duplicate rows 0-63
        xTh = np.empty((128, BT), dtype=F16NP)
        xTh[:D] = np.transpose(xs, (2, 1, 0)).reshape(D, BT)
        xTh[D:] = xTh[:D]
        ins.append({
            "xT": xTh, "w1": w1h, "w2": w2h, "b1c": b1ch, "b2c": b2ch,
            "hm": hmh, "hf": hfh, "hl": hlh,
        })
    return ins


def kernel(x, W1, b1, W2, b2, H_first, H_mid, H_last):
    from concourse.bass_utils import run_bass_kernel_spmd

    if "nc" not in _CACHE:
        _CACHE["nc"] = _build()
    nc = _CACHE["nc"]

    in_maps = _prep_inputs(x, W1, b1, W2, b2, H_first, H_mid, H_last)
    res = run_bass_kernel_spmd(nc, in_maps, core_ids=list(range(NCORES)))
    # out[b_in_tile, tile] per core -> flat [BC] with index tile*128 + b
    outs = [
        np.asarray(res.results[c]["out"]).T.reshape(BC) for c in range(NCORES)
    ]
    full = np.concatenate(outs, axis=0).astype(np.float64)
    return (full / SCALE**NSCALED).astype(np.float32)
